# revision 29
# baseline (speedup 1.0000x reference)
"""DeepSeek-MoE FFN (8 routed experts, top-2, SwiGLU, shared expert) on 8
Trainium2 NeuronCores.

Strategy: token-parallel, host-routed sparse. Each core takes N/8 = 2048
tokens. Routing (gate logits, top-2, softmax) is computed on host in fp64
(0.06% of total FLOPs) and shipped as per-expert index lists + a combine
weight table; the device kernel is a pure gather -> SwiGLU expert ->
scale -> scatter-add pipeline plus a dense shared expert, with no
on-device routing phase. Expert matmuls run in bf16 with fp32 PSUM
accumulation. Per-expert capacities are sized to the actual routed counts
(max over cores, rounded up to 64) so padded compute is minimal.

Device timeline: the shared expert (needs only streamed activations +
its weights) starts within ~2us; routed experts follow back-to-back with
weights/gathers double-buffered on parallel DMA queues, keeping the PE
array ~97% busy. Outputs accumulate directly in the output DRAM tensor:
shared writes rows, each expert scatter-adds its scaled slots.

Per-core layouts (host-prepped, d-chunked so every DMA line is contiguous):
  xtb  [128, 8, 2048] bf16  xtb[p, c, t] = x[t, c*128+p]   (shared expert)
  xb   [2064, 1024]   bf16  row-major tokens + 16 zero pad rows (gathers)
  wg   [9, 12, 128, 8, 128] bf16  wg[u, fc, p, c, f] = Wg_u[fc*128+f, c*128+p]
  wu   same layout for the up projection
  wd   [9, 12, 128, 1024]   bf16  wd[u, fc, p, d]    = Wd_u[d, fc*128+p]
  (unit 8 is the shared expert)
  idx  [128, E, CWMAX] int16  per-expert slot->token ids, ucode layout
  combR[E, 2064, 64]  f32   combine weight per (expert, token), 64-wide
  out  [2064, 1024]   f32   row-major output (+16 scratch pad rows)
"""

import sys

if '/opt/trn_rl_repo' not in sys.path:
    sys.path.insert(0, '/opt/trn_rl_repo')

from contextlib import ExitStack

import numpy as np
import ml_dtypes

import concourse.bass as bass
import concourse.tile as tile
import concourse.mybir as mybir
from concourse.alu_op_type import AluOpType
from concourse.vector_clock import ScopedClock

bf16 = ml_dtypes.bfloat16
F32 = mybir.dt.float32
BF = mybir.dt.bfloat16
AF = mybir.ActivationFunctionType
AX = mybir.AxisListType

# ---------------------------------------------------------------------------
# TileContext tail-drain fix: the stock exit emits one Drain carrying a sem
# wait per live logical proc, but walrus only accepts a single sync wait per
# SP instruction. Split the waits across preceding sync nops.
_MAX_WAITS = 1


def _patched_drain_and_barrier(self, tick_clock, wait_clock):
    nc = self.nc
    probe = nc.sync.nop()
    wait_clock.add_sem_waits(probe.ins, ScopedClock({None: tick_clock.global_clock}))
    si = probe.ins.sync_info
    waits = list(si.on_wait) if si is not None else []
    if len(waits) > _MAX_WAITS:
        probe.ins.sync_info = mybir.SyncInfo(on_wait=waits[:_MAX_WAITS], on_update=[])
        for k in range(_MAX_WAITS, len(waits), _MAX_WAITS):
            n = nc.sync.nop()
            n.ins.sync_info = mybir.SyncInfo(
                on_wait=waits[k:k + _MAX_WAITS], on_update=[]
            )
    nc.sync.drain()
    nc.all_engine_barrier()
    assert self.sems is not None
    popped = nc._tile_sem_poison_stack.pop()
    assert popped is self._sem_poison
    nc.clear_and_free_semaphores(list(self.sems.allocated().values()))
    nc.all_engine_barrier()


tile.TileContext._drain_and_barrier = _patched_drain_and_barrier

# ---------------------------------------------------------------------------
# This walrus build accepts only ONE sync wait per instruction. Hoist extra
# waits onto standalone same-engine NoOps placed immediately before.
_WSPLIT_ID = [0]


def _split_multi_waits(nc):
    for f in nc.m.functions:
        for bb in f.blocks:
            out = []
            changed = False
            for inst in bb.instructions:
                si = getattr(inst, 'sync_info', None)
                if si is not None and si.on_wait and len(si.on_wait) > 1:
                    changed = True
                    waits = list(si.on_wait)
                    for w in waits[:-1]:
                        n = mybir.InstNoOp(
                            name=f"I-wsplit{_WSPLIT_ID[0]}", ins=[], outs=[])
                        _WSPLIT_ID[0] += 1
                        n.engine = inst.engine
                        n.sync_info = mybir.SyncInfo(on_wait=[w], on_update=[])
                        out.append(n)
                    inst.sync_info = mybir.SyncInfo(
                        on_wait=[waits[-1]],
                        on_update=list(si.on_update or []))
                out.append(inst)
            if changed:
                bb.instructions = out


P = 128


def _st_chunks(cap, cw=512):
    out = []
    s0 = 0
    while s0 < cap:
        w = min(cw, cap - s0)
        out.append((s0, w))
        s0 += w
    return out


def build_moe_hostroute(DC=8, FC=12, E=8, NLOC=2048, CAPS=(640,) * 8,
                        split_waits=True, repeat=1):
    """Host-routed sparse MoE kernel.

    DC: contraction chunks (D = DC*128); FC: half-ffn chunks (HALF = FC*128);
    E: routed experts; NLOC: tokens per core; CAPS: per-expert capacity
    (multiple of 64; >= actual routed count on every core).
    """
    from concourse import library_config

    UNITS = E + 1
    D = DC * P
    CWS = [c // 16 for c in CAPS]
    NSTS = [-(-c // P) for c in CAPS]     # ysc second dim (ceil cap/128)
    NSTMAX = max(NSTS)
    CAPG = NSTMAX * P                     # fixed gather size (pad -> zeros)
    CWMAX = CAPG // 16

    nc = bass.Bass(target_bir_lowering=False)
    xtb = nc.declare_dram_parameter("xtb", [P, DC, NLOC], BF, isOutput=False)
    xb = nc.declare_dram_parameter("xb", [NLOC + 16, D], BF, isOutput=False)
    wg = nc.declare_dram_parameter("wg", [UNITS, FC, P, DC, P], BF, isOutput=False)
    wu = nc.declare_dram_parameter("wu", [UNITS, FC, P, DC, P], BF, isOutput=False)
    wd = nc.declare_dram_parameter("wd", [UNITS, FC, P, D], BF, isOutput=False)
    idxp = nc.declare_dram_parameter(
        "idx", [P, E, CWMAX], mybir.dt.int16, isOutput=False)
    combR = nc.declare_dram_parameter(
        "combR", [E, NLOC + 16, 64], F32, isOutput=False)
    outp = nc.declare_dram_parameter("out", [NLOC + 16, D], F32, isOutput=True)

    EORDER = sorted(range(E), key=lambda e: -CAPS[e])
    _LAST = EORDER[-1]

    with tile.TileContext(nc) as tc:
      _regvals = {CAPG, 256} | {c - 256 for c in CAPS}
      _regvals |= {min(P, CAPS[_LAST] - g * P)
                   for g in range(2, NSTS[_LAST])}
      cap_regs = {c: nc.gpsimd.to_reg(c) for c in sorted(_regvals)}
      # load the gpsimd ucode library ONCE per NEFF (not per rep: it is
      # expensive on hardware, and per-rep reloads would also pollute the
      # repeat-slope timing methodology)
      lib_mlp = nc.gpsimd.load_library(library_config.mlp)
      for _rep in range(repeat):
        with ExitStack() as ctx:
            cpool = ctx.enter_context(tc.tile_pool(name="const", bufs=1))
            wpool = ctx.enter_context(tc.tile_pool(name="wpool", bufs=2))
            wdpool = ctx.enter_context(tc.tile_pool(name="wdpool", bufs=1))
            xgpool = ctx.enter_context(tc.tile_pool(name="xgpool", bufs=2))
            cgpool = ctx.enter_context(tc.tile_pool(name="cgpool", bufs=2))
            hpool = ctx.enter_context(tc.tile_pool(name="hpool", bufs=1))
            spool = ctx.enter_context(tc.tile_pool(name="spool", bufs=2))
            ypool = ctx.enter_context(tc.tile_pool(name="ypool", bufs=1))
            gpsum = ctx.enter_context(
                tc.tile_pool(name="gpsum", bufs=2, space="PSUM"))
            upsum = ctx.enter_context(
                tc.tile_pool(name="upsum", bufs=2, space="PSUM"))
            ypsum = ctx.enter_context(
                tc.tile_pool(name="ypsum", bufs=2, space="PSUM"))

            idx_sb = cpool.tile([P, E, CWMAX], mybir.dt.int16)
            nc.sync.dma_start(idx_sb[:], idxp[:, :, :])

            def load_unit_gu(u):
                # ALL weight copies go on the SP queue: any DMA issue op on
                # the Act queue can stall on DMA ring credits at expert
                # boundaries (scatter + wd transfers congest the rings) and
                # silus queued behind it would stall the PE via gpsum
                # slot recycling.
                wg_sb = wpool.tile([P, FC, DC, P], BF, tag="wg")
                wu_sb = wpool.tile([P, FC, DC, P], BF, tag="wu")
                for fc in range(FC):
                    nc.sync.dma_start(wg_sb[:, fc], wg[u, fc])
                    nc.sync.dma_start(wu_sb[:, fc], wu[u, fc])
                return wg_sb, wu_sb

            def load_unit_d(u):
                # down weights: single-buffered, issued at the END of the
                # previous unit's body, and ONLY on the SP queue. The
                # slot-wait (previous wd release = its last down matmul)
                # blocks the issuing queue head until that unit finishes;
                # on the Act queue that would jam the next unit's silus
                # (which recycle the g/u PSUM slots) and stall the PE ~19us
                # per expert. The SP queue carries nothing latency-critical
                # at that point, so the block is harmless there.
                wd_sb = wdpool.tile([P, FC, D], BF, tag="wd")
                for fc in range(FC):
                    nc.sync.dma_start(wd_sb[:, fc], wd[u, fc])
                return wd_sb

            def issue_gathers(e):
                # dma_gather needs num_idxs % 128 == 0: always gather the
                # fixed CAPG (pad entries hit the zero dummy row, keeping
                # every tile one size); compute + scatter cover only the
                # tight 16-granular CAPS[e].
                xg_sb = xgpool.tile([P, DC, CAPG], BF, tag="xg",
                                    name=f"xg_{_rep}_{e}")
                g1 = nc.gpsimd.dma_gather(
                    xg_sb[:], xb[:, :], idx_sb[:, e, :],
                    num_idxs=CAPG, num_idxs_reg=cap_regs[CAPG], elem_size=D,
                    transpose=True)
                tile.add_dep_helper(g1.ins, lib_mlp.ins, reason="mlp lib")
                cg_sb = cgpool.tile([P, NSTMAX, 64], F32, tag="cg",
                                    name=f"cg_{_rep}_{e}")
                g2 = nc.gpsimd.dma_gather(
                    cg_sb[:], combR[e], idx_sb[:, e, :],
                    num_idxs=CAPG, num_idxs_reg=cap_regs[CAPG], elem_size=64,
                    transpose=False)
                tile.add_dep_helper(g2.ins, lib_mlp.ins, reason="mlp lib")
                return xg_sb, cg_sb

            def gu_sweep(wg_sb, wu_sb, rhs_fn, width):
                # 512-wide moving dim: each fc chain fills a full 2KB PSUM
                # bank, halving the PE instruction count vs 256-wide tiles.
                hs_sb = hpool.tile([P, FC, 512], BF, tag="hs")
                for fc in range(FC):
                    ps_g = gpsum.tile([P, 512], F32, tag="pg")
                    ps_u = upsum.tile([P, 512], F32, tag="pu")
                    for c in range(DC):
                        nc.tensor.matmul(
                            ps_g[:, 0:width], wg_sb[:, fc, c, :], rhs_fn(c),
                            start=(c == 0), stop=(c == DC - 1))
                    for c in range(DC):
                        nc.tensor.matmul(
                            ps_u[:, 0:width], wu_sb[:, fc, c, :], rhs_fn(c),
                            start=(c == 0), stop=(c == DC - 1))
                    sg_t = spool.tile([P, 512], F32, tag="sg")
                    nc.scalar.activation(
                        sg_t[:, 0:width], ps_g[:, 0:width], AF.Silu)
                    nc.vector.tensor_tensor(
                        hs_sb[:, fc, 0:width], sg_t[:, 0:width],
                        ps_u[:, 0:width], op=AluOpType.mult)
                return hs_sb

            def down_sub(hs_sb, wd_sb, sub, w=P):
                yp = ypsum.tile([P, D], F32, tag="yp")
                dw = 512
                for half in range(D // dw):
                    for fc in range(FC):
                        nc.tensor.matmul(
                            yp[0:w, half * dw:(half + 1) * dw],
                            hs_sb[:, fc, sub * P:sub * P + w],
                            wd_sb[:, fc, half * dw:(half + 1) * dw],
                            start=(fc == 0), stop=(fc == FC - 1))
                return yp

            # ---- shared expert (unit E), streamed activations ----
            # first chunk is 256-wide so the PE starts ~3us earlier; the
            # first fc of the gate/up weights is interleaved between the
            # first two activation tiles on the SP ring for the same reason
            SH_CHUNKS = [(512 * i, 512) for i in range(NLOC // 512)]
            nsh = len(SH_CHUNKS)
            with ExitStack() as sctx:
                stpool = sctx.enter_context(
                    tc.tile_pool(name="stpool", bufs=2))

                def issue_xt(i):
                    s0, w = SH_CHUNKS[i]
                    t = stpool.tile([P, DC, 512], BF, tag="xt",
                                    name=f"xt_{_rep}_{i}")
                    nc.sync.dma_start(t[:, :, 0:w], xtb[:, :, s0:s0 + w])
                    return t

                # startup only: wu rides the idle Act ring so the fc
                # supply rate is 2x the PE's consumption rate (no
                # boundary-congestion hazard exists at t=0)
                xt_tiles = [issue_xt(0)]
                wgE = wpool.tile([P, FC, DC, P], BF, tag="wg")
                wuE = wpool.tile([P, FC, DC, P], BF, tag="wu")
                nc.sync.dma_start(wgE[:, 0], wg[E, 0])
                nc.sync.dma_start(wuE[:, 0], wu[E, 0])
                xt_tiles.append(issue_xt(1))
                for fc in range(1, FC):
                    nc.sync.dma_start(wgE[:, fc], wg[E, fc])
                    nc.sync.dma_start(wuE[:, fc], wu[E, fc])
                wdE = load_unit_d(E)
                w_next = None
                for i, (s0, w) in enumerate(SH_CHUNKS):
                    xcur = xt_tiles[i % 2]
                    hs_sb = gu_sweep(
                        wgE, wuE, lambda c, x=xcur, ww=w: x[:, c, 0:ww], w)
                    if i == 1:
                        # second chunk, not first: the startup DMA server
                        # must feed xtb + shared weights before anything else
                        w_next = load_unit_gu(EORDER[0])
                        xg_cur, cg_cur = issue_gathers(EORDER[0])
                    for sub in range(w // P):
                        yp = down_sub(hs_sb, wdE, sub)
                        ysh = spool.tile([P, D], F32, tag="ysh")
                        nc.scalar.copy(ysh[:], yp[:])
                        r0 = s0 + sub * P
                        nc.sync.dma_start(outp[r0:r0 + P, :], ysh[:])
                    if i + 2 < nsh:
                        xt_tiles[i % 2] = issue_xt(i + 2)
            wd_next = load_unit_d(EORDER[0])  # end-of-body: see load_unit_d

            # ---- routed experts, largest capacity first so the final
            # expert has the smallest tail scatter ----
            for ei in range(E):
                e = EORDER[ei]
                cap = CAPS[e]
                last = ei + 1 >= E
                wg_sb, wu_sb = w_next
                wd_sb = wd_next
                xg_sb, cg_sb = xg_cur, cg_cur
                ysc = ypool.tile([P, NSTMAX, D], F32, tag="ysc")
                first = True
                for (s0, sw) in _st_chunks(cap):
                    hs_sb = gu_sweep(
                        wg_sb, wu_sb,
                        lambda c, x=xg_sb, a=s0, b=sw: x[:, c, a:a + b], sw)
                    if first and not last:
                        # mid-body prefetch: issue after the first chunk so
                        # the slot-wait can't block queue heads at e's start
                        w_next = load_unit_gu(EORDER[ei + 1])
                        xg_cur, cg_cur = issue_gathers(EORDER[ei + 1])
                    nsub = -(-sw // P)
                    for sub in range(nsub):
                        w = min(P, sw - sub * P)
                        gsub = s0 // P + sub
                        yp = down_sub(hs_sb, wd_sb, sub, w=w)
                        nc.vector.tensor_scalar(
                            ysc[0:w, gsub, :], yp[0:w, :],
                            cg_sb[0:w, gsub, 0:1], None, op0=AluOpType.mult)
                        if gsub == 1:
                            # scatter the first 256 slots as soon as they
                            # are scaled: spreads the RMW DMA away from the
                            # expert boundary and off the drain tail
                            scA = nc.gpsimd.dma_scatter_add(
                                outp[:, :], ysc[:, 0:2, :],
                                idx_sb[:, e, 0:16],
                                num_idxs=256, num_idxs_reg=cap_regs[256],
                                elem_size=D)
                            tile.add_dep_helper(scA.ins, lib_mlp.ins,
                                                reason="mlp lib")
                        elif last and gsub >= 2:
                            # final expert: scatter each 128-slot sub as it
                            # completes to shrink the drain tail
                            n = min(P, cap - gsub * P)
                            scP = nc.gpsimd.dma_scatter_add(
                                outp[:, :], ysc[:, gsub:gsub + 1, :],
                                idx_sb[:, e, gsub * 8:gsub * 8 + n // 16],
                                num_idxs=n, num_idxs_reg=cap_regs[n],
                                elem_size=D)
                            tile.add_dep_helper(scP.ins, lib_mlp.ins,
                                                reason="mlp lib")
                    if first:
                        first = False
                if not last:
                    scB = nc.gpsimd.dma_scatter_add(
                        outp[:, :], ysc[:, 2:NSTS[e], :],
                        idx_sb[:, e, 16:CWS[e]],
                        num_idxs=cap - 256, num_idxs_reg=cap_regs[cap - 256],
                        elem_size=D)
                    tile.add_dep_helper(scB.ins, lib_mlp.ins,
                                        reason="mlp lib")
                    wd_next = load_unit_d(EORDER[ei + 1])

    from concourse.library_overlay import lower_extended_insts
    lower_extended_insts(nc)
    if split_waits:
        _split_multi_waits(nc)
    return nc


# ---------------------------------------------------------------------------
# Host side


def _prep_weight_gu(w, DC, FC):
    # w [HALF, D] -> [FC, 128, DC, 128]: out[fc, p, c, f] = w[fc*128+f, c*128+p]
    wt = w.T.reshape(DC, P, FC, P).transpose(2, 1, 0, 3)
    return np.ascontiguousarray(wt.astype(bf16))


def _prep_weight_d(w, DC, FC):
    # w [D, HALF] -> [FC, 128, D]: out[fc, p, d] = w[d, fc*128+p]
    wt = w.T.reshape(FC, P, DC * P)
    return np.ascontiguousarray(wt.astype(bf16))


_BUILT = {}
_LAST_CAPS = None


def _get_built(key, **kw):
    if key not in _BUILT:
        _BUILT[key] = build_moe_hostroute(**kw)
    return _BUILT[key]


def _host_route(xf, gate_w, NCORES, NLOC, E):
    """fp64 routing + balanced token->core assignment.

    Routing (gate logits, top-2, softmax) runs in fp64 numpy. Tokens are
    then assigned to cores greedily to balance per-(core, expert) counts
    (penalizing any count crossing the 512 boundary, which would cost an
    extra 128-slot down-projection sub-tile), so per-expert capacities are
    minimal. Returns (perm, CAPS, idx_maps, combR_maps): core ci owns
    tokens perm[ci*NLOC:(ci+1)*NLOC].
    """
    N = xf.shape[0]
    logits = xf.astype(np.float64) @ gate_w.astype(np.float64).T   # [N, E]
    top2 = np.argsort(-logits, axis=1, kind='stable')[:, :2]       # [N, 2]
    tv = np.take_along_axis(logits, top2, axis=1)
    ex = np.exp(tv - tv[:, 0:1])
    w12 = ex / ex.sum(axis=1, keepdims=True)                       # [N, 2]

    # --- greedy balanced assignment ---
    glob = np.bincount(top2.ravel(), minlength=E)
    prio = np.maximum(glob[top2[:, 0]], glob[top2[:, 1]])
    order = np.argsort(-prio, kind='stable')
    counts = [[0] * E for _ in range(NCORES)]
    loads = [0] * NCORES
    assign = np.empty(N, dtype=np.int64)
    t2l = top2.tolist()
    for t in order.tolist():
        e1, e2 = t2l[t]
        best, bestscore = -1, None
        for c in range(NCORES):
            if loads[c] >= NLOC:
                continue
            cc = counts[c]
            n1, n2 = cc[e1] + 1, cc[e2] + 1
            score = ((n1 > 512) + (n2 > 512),
                     n1 if n1 > n2 else n2, n1 + n2, loads[c])
            if bestscore is None or score < bestscore:
                bestscore, best = score, c
        assign[t] = best
        counts[best][e1] += 1
        counts[best][e2] += 1
        loads[best] += 1
    perm = np.argsort(assign, kind='stable')

    counts = np.asarray(counts)
    CAPS = tuple(int(max(64, -(-counts[:, e].max() // 16) * 16))
                 for e in range(E))
    CWS = [c // 16 for c in CAPS]
    NSTMAX = max(-(-c // P) for c in CAPS)
    CWMAX = NSTMAX * 8          # gathers always fetch NSTMAX*128 entries

    idx_maps, combR_maps = [], []
    for ci in range(NCORES):
        toks = perm[ci * NLOC:(ci + 1) * NLOC]
        t2 = top2[toks]
        wl = w12[toks]
        idxa = np.full((P, E, CWMAX), NLOC, dtype=np.int16)
        cR = np.zeros((E, NLOC + 16, 64), dtype=np.float32)
        for e in range(E):
            rows, cols = np.nonzero(t2 == e)
            assert len(rows) <= CAPS[e], (e, len(rows), CAPS[e])
            cR[e, rows, :] = wl[rows, cols].astype(np.float32)[:, None]
            arr = np.full(CAPS[e], NLOC, dtype=np.int16)
            arr[:len(rows)] = rows.astype(np.int16)
            idci = arr.reshape(CWS[e], 16).T                        # [16, CW]
            idxa[:, e, :CWS[e]] = np.tile(idci, (8, 1))
        idx_maps.append(idxa)
        combR_maps.append(cR)
    return perm, CAPS, idx_maps, combR_maps


def prepare(x, gate_w, w_up, w_down, sg_gate, sg_up, sg_down):
    """Build (nc, in_maps, meta) for the 8-core SPMD launch."""
    global _LAST_CAPS
    B, T, D = x.shape
    E = gate_w.shape[0]
    FFN = w_up.shape[1]
    HALF = FFN // 2
    DC = D // P
    FC = HALF // P
    N = B * T
    NCORES = 8
    NLOC = N // NCORES

    xf = np.ascontiguousarray(x.reshape(N, D))
    perm, CAPS, idx_maps, combR_maps = _host_route(
        xf, gate_w, NCORES, NLOC, E)
    _LAST_CAPS = CAPS
    xf = xf[perm]

    nc = _get_built((DC, FC, E, NLOC, CAPS),
                    DC=DC, FC=FC, E=E, NLOC=NLOC, CAPS=CAPS)

    UNITS = E + 1
    wg_all = np.empty((UNITS, FC, P, DC, P), dtype=bf16)
    wu_all = np.empty((UNITS, FC, P, DC, P), dtype=bf16)
    wd_all = np.empty((UNITS, FC, P, D), dtype=bf16)
    for u in range(E):
        wg_all[u] = _prep_weight_gu(w_up[u, :HALF], DC, FC)
        wu_all[u] = _prep_weight_gu(w_up[u, HALF:], DC, FC)
        wd_all[u] = _prep_weight_d(w_down[u], DC, FC)
    wg_all[E] = _prep_weight_gu(sg_gate, DC, FC)
    wu_all[E] = _prep_weight_gu(sg_up, DC, FC)
    wd_all[E] = _prep_weight_d(sg_down, DC, FC)

    in_maps = []
    for ci in range(NCORES):
        xc = xf[ci * NLOC:(ci + 1) * NLOC]
        xt = xc.T.reshape(DC, P, NLOC).transpose(1, 0, 2)
        xtb = np.ascontiguousarray(xt.astype(bf16))
        xbp = np.zeros((NLOC + 16, D), dtype=bf16)
        xbp[:NLOC] = xc.astype(bf16)
        in_maps.append({
            "xtb": xtb, "xb": xbp,
            "wg": wg_all, "wu": wu_all, "wd": wd_all,
            "idx": idx_maps[ci], "combR": combR_maps[ci],
        })

    return nc, in_maps, (B, T, D, NLOC, NCORES, perm)


def postprocess(results, meta):
    B, T, D, NLOC, NCORES, perm = meta
    cat = np.concatenate(
        [results[ci]["out"][0:NLOC] for ci in range(NCORES)], axis=0)
    out = np.empty_like(cat)
    out[perm] = cat
    return out.reshape(B, T, D).astype(np.float32)


def kernel(x, gate_w, w_up, w_down, sg_gate, sg_up, sg_down):
    from concourse.bass_utils import run_bass_kernel_spmd

    nc, in_maps, meta = prepare(
        x, gate_w, w_up, w_down, sg_gate, sg_up, sg_down)
    r = run_bass_kernel_spmd(nc, in_maps, core_ids=list(range(meta[4])))
    return postprocess(r.results, meta)


# revision 31
# speedup vs baseline: 1.2035x; 1.2035x over previous
"""DeepSeek-MoE FFN (8 routed experts, top-2, SwiGLU, shared expert) on 8
Trainium2 NeuronCores.

Strategy: token-parallel, host-routed sparse. Routing (gate logits,
top-2, softmax) is computed on host in fp64 (0.06% of total FLOPs);
tokens are then assigned to cores by a balanced greedy pass that
minimizes per-(core, expert) counts, and shipped as per-expert index
lists + a combine weight table. The device kernel is a pure gather ->
SwiGLU expert -> scale -> scatter-add pipeline plus a dense shared
expert, with no on-device routing phase. Expert matmuls run in bf16
with fp32 PSUM accumulation, 512-wide moving tiles. Per-expert
capacities are sized to the actual routed counts (max over cores,
rounded up to 16) so padded compute is minimal.

Device timeline: the shared expert (needs only streamed activations +
its weights) starts within ~5us; routed experts follow back-to-back with
weights/gathers prefetched on the SP DMA queue (kept off the Act queue,
whose silus recycle the PSUM slots the PE waits on), keeping the PE
array ~97% busy. Outputs accumulate directly in the output DRAM tensor:
shared writes rows, each expert scatter-adds its scaled slots
(first 256 early, remainder at expert end).

Per-core layouts (host-prepped, d-chunked so every DMA line is contiguous):
  xtb  [128, 8, 2048] bf16  xtb[p, c, t] = x[t, c*128+p]   (shared expert)
  xb   [2064, 1024]   bf16  row-major tokens + 16 zero pad rows (gathers)
  wg   [9, 12, 128, 8, 128] bf16  wg[u, fc, p, c, f] = Wg_u[fc*128+f, c*128+p]
  wu   same layout for the up projection
  wd   [9, 12, 128, 1024]   bf16  wd[u, fc, p, d]    = Wd_u[d, fc*128+p]
  (unit 8 is the shared expert)
  idx  [128, E, CWMAX] int16  per-expert slot->token ids, ucode layout
  combR[E, 2064, 64]  f32   combine weight per (expert, token), 64-wide
  out  [2064, 1024]   f32   row-major output (+16 scratch pad rows)
"""

import sys

if '/opt/trn_rl_repo' not in sys.path:
    sys.path.insert(0, '/opt/trn_rl_repo')

from contextlib import ExitStack

import numpy as np
import ml_dtypes

import concourse.bass as bass
import concourse.tile as tile
import concourse.mybir as mybir
from concourse.alu_op_type import AluOpType
from concourse.vector_clock import ScopedClock

bf16 = ml_dtypes.bfloat16
F32 = mybir.dt.float32
BF = mybir.dt.bfloat16
AF = mybir.ActivationFunctionType
AX = mybir.AxisListType

# ---------------------------------------------------------------------------
# TileContext tail-drain fix: the stock exit emits one Drain carrying a sem
# wait per live logical proc, but walrus only accepts a single sync wait per
# SP instruction. Split the waits across preceding sync nops.
_MAX_WAITS = 1


def _patched_drain_and_barrier(self, tick_clock, wait_clock):
    nc = self.nc
    probe = nc.sync.nop()
    wait_clock.add_sem_waits(probe.ins, ScopedClock({None: tick_clock.global_clock}))
    si = probe.ins.sync_info
    waits = list(si.on_wait) if si is not None else []
    if len(waits) > _MAX_WAITS:
        probe.ins.sync_info = mybir.SyncInfo(on_wait=waits[:_MAX_WAITS], on_update=[])
        for k in range(_MAX_WAITS, len(waits), _MAX_WAITS):
            n = nc.sync.nop()
            n.ins.sync_info = mybir.SyncInfo(
                on_wait=waits[k:k + _MAX_WAITS], on_update=[]
            )
    nc.sync.drain()
    nc.all_engine_barrier()
    assert self.sems is not None
    popped = nc._tile_sem_poison_stack.pop()
    assert popped is self._sem_poison
    nc.clear_and_free_semaphores(list(self.sems.allocated().values()))
    nc.all_engine_barrier()


tile.TileContext._drain_and_barrier = _patched_drain_and_barrier

# ---------------------------------------------------------------------------
# This walrus build accepts only ONE sync wait per instruction. Hoist extra
# waits onto standalone same-engine NoOps placed immediately before.
_WSPLIT_ID = [0]


def _split_multi_waits(nc):
    for f in nc.m.functions:
        for bb in f.blocks:
            out = []
            changed = False
            for inst in bb.instructions:
                si = getattr(inst, 'sync_info', None)
                if si is not None and si.on_wait and len(si.on_wait) > 1:
                    changed = True
                    waits = list(si.on_wait)
                    for w in waits[:-1]:
                        n = mybir.InstNoOp(
                            name=f"I-wsplit{_WSPLIT_ID[0]}", ins=[], outs=[])
                        _WSPLIT_ID[0] += 1
                        n.engine = inst.engine
                        n.sync_info = mybir.SyncInfo(on_wait=[w], on_update=[])
                        out.append(n)
                    inst.sync_info = mybir.SyncInfo(
                        on_wait=[waits[-1]],
                        on_update=list(si.on_update or []))
                out.append(inst)
            if changed:
                bb.instructions = out


P = 128


def _st_chunks(cap, cw=512):
    out = []
    s0 = 0
    while s0 < cap:
        w = min(cw, cap - s0)
        out.append((s0, w))
        s0 += w
    return out


def build_moe_hostroute(DC=8, FC=12, E=8, NLOC=2048, CAPS=(640,) * 8,
                        split_waits=True, repeat=1):
    """Host-routed sparse MoE kernel.

    DC: contraction chunks (D = DC*128); FC: half-ffn chunks (HALF = FC*128);
    E: routed experts; NLOC: tokens per core; CAPS: per-expert capacity
    (multiple of 64; >= actual routed count on every core).
    """
    from concourse import library_config

    UNITS = E + 1
    D = DC * P
    CWS = [c // 16 for c in CAPS]
    NSTS = [-(-c // P) for c in CAPS]     # ysc second dim (ceil cap/128)
    NSTMAX = max(NSTS)
    CAPG = NSTMAX * P                     # fixed gather size (pad -> zeros)
    CWMAX = CAPG // 16

    nc = bass.Bass(target_bir_lowering=False)
    xtb = nc.declare_dram_parameter("xtb", [P, DC, NLOC], BF, isOutput=False)
    xb = nc.declare_dram_parameter("xb", [NLOC + 16, D], BF, isOutput=False)
    wg = nc.declare_dram_parameter("wg", [UNITS, FC, P, DC, P], BF, isOutput=False)
    wu = nc.declare_dram_parameter("wu", [UNITS, FC, P, DC, P], BF, isOutput=False)
    wd = nc.declare_dram_parameter("wd", [UNITS, FC, P, D], BF, isOutput=False)
    idxp = nc.declare_dram_parameter(
        "idx", [P, E, CWMAX], mybir.dt.int16, isOutput=False)
    combR = nc.declare_dram_parameter(
        "combR", [E, NLOC + 16, 64], F32, isOutput=False)
    outp = nc.declare_dram_parameter("out", [NLOC + 16, D], F32, isOutput=True)

    EORDER = list(range(E))
    _LAST = EORDER[-1]

    with tile.TileContext(nc) as tc:
      _regvals = {CAPG, 256} | {c - 256 for c in CAPS}
      _regvals |= {min(P, CAPS[_LAST] - g * P)
                   for g in range(2, NSTS[_LAST])}
      cap_regs = {c: nc.gpsimd.to_reg(c) for c in sorted(_regvals)}
      # load the gpsimd ucode library ONCE per NEFF (not per rep: it is
      # expensive on hardware, and per-rep reloads would also pollute the
      # repeat-slope timing methodology)
      lib_mlp = nc.gpsimd.load_library(library_config.mlp)
      for _rep in range(repeat):
        with ExitStack() as ctx:
            cpool = ctx.enter_context(tc.tile_pool(name="const", bufs=1))
            wpool = ctx.enter_context(tc.tile_pool(name="wpool", bufs=2))
            wdpool = ctx.enter_context(tc.tile_pool(name="wdpool", bufs=1))
            xgpool = ctx.enter_context(tc.tile_pool(name="xgpool", bufs=2))
            cgpool = ctx.enter_context(tc.tile_pool(name="cgpool", bufs=2))
            hpool = ctx.enter_context(tc.tile_pool(name="hpool", bufs=1))
            spool = ctx.enter_context(tc.tile_pool(name="spool", bufs=2))
            ypool = ctx.enter_context(tc.tile_pool(name="ypool", bufs=1))
            gpsum = ctx.enter_context(
                tc.tile_pool(name="gpsum", bufs=2, space="PSUM"))
            upsum = ctx.enter_context(
                tc.tile_pool(name="upsum", bufs=2, space="PSUM"))
            ypsum = ctx.enter_context(
                tc.tile_pool(name="ypsum", bufs=2, space="PSUM"))

            idx_sb = cpool.tile([P, E, CWMAX], mybir.dt.int16)
            nc.sync.dma_start(idx_sb[:], idxp[:, :, :])

            def load_unit_gu(u):
                # ALL weight copies go on the SP queue: any DMA issue op on
                # the Act queue can stall on DMA ring credits at expert
                # boundaries (scatter + wd transfers congest the rings) and
                # silus queued behind it would stall the PE via gpsum
                # slot recycling.
                wg_sb = wpool.tile([P, FC, DC, P], BF, tag="wg")
                wu_sb = wpool.tile([P, FC, DC, P], BF, tag="wu")
                for fc in range(FC):
                    nc.sync.dma_start(wg_sb[:, fc], wg[u, fc])
                    nc.sync.dma_start(wu_sb[:, fc], wu[u, fc])
                return wg_sb, wu_sb

            def load_unit_d(u):
                # down weights: single-buffered, issued at the END of the
                # previous unit's body, and ONLY on the SP queue. The
                # slot-wait (previous wd release = its last down matmul)
                # blocks the issuing queue head until that unit finishes;
                # on the Act queue that would jam the next unit's silus
                # (which recycle the g/u PSUM slots) and stall the PE ~19us
                # per expert. The SP queue carries nothing latency-critical
                # at that point, so the block is harmless there.
                wd_sb = wdpool.tile([P, FC, D], BF, tag="wd")
                for fc in range(FC):
                    nc.sync.dma_start(wd_sb[:, fc], wd[u, fc])
                return wd_sb

            def issue_gathers(e):
                # dma_gather needs num_idxs % 128 == 0: always gather the
                # fixed CAPG (pad entries hit the zero dummy row, keeping
                # every tile one size); compute + scatter cover only the
                # tight 16-granular CAPS[e].
                xg_sb = xgpool.tile([P, DC, CAPG], BF, tag="xg",
                                    name=f"xg_{_rep}_{e}")
                g1 = nc.gpsimd.dma_gather(
                    xg_sb[:], xb[:, :], idx_sb[:, e, :],
                    num_idxs=CAPG, num_idxs_reg=cap_regs[CAPG], elem_size=D,
                    transpose=True)
                tile.add_dep_helper(g1.ins, lib_mlp.ins, reason="mlp lib")
                cg_sb = cgpool.tile([P, NSTMAX, 64], F32, tag="cg",
                                    name=f"cg_{_rep}_{e}")
                g2 = nc.gpsimd.dma_gather(
                    cg_sb[:], combR[e], idx_sb[:, e, :],
                    num_idxs=CAPG, num_idxs_reg=cap_regs[CAPG], elem_size=64,
                    transpose=False)
                tile.add_dep_helper(g2.ins, lib_mlp.ins, reason="mlp lib")
                return xg_sb, cg_sb

            def gu_sweep(wg_sb, wu_sb, rhs_fn, width):
                # 512-wide moving dim: each fc chain fills a full 2KB PSUM
                # bank, halving the PE instruction count vs 256-wide tiles.
                hs_sb = hpool.tile([P, FC, 512], BF, tag="hs")
                for fc in range(FC):
                    ps_g = gpsum.tile([P, 512], F32, tag="pg")
                    ps_u = upsum.tile([P, 512], F32, tag="pu")
                    for c in range(DC):
                        nc.tensor.matmul(
                            ps_g[:, 0:width], wg_sb[:, fc, c, :], rhs_fn(c),
                            start=(c == 0), stop=(c == DC - 1))
                    for c in range(DC):
                        nc.tensor.matmul(
                            ps_u[:, 0:width], wu_sb[:, fc, c, :], rhs_fn(c),
                            start=(c == 0), stop=(c == DC - 1))
                    sg_t = spool.tile([P, 512], F32, tag="sg")
                    nc.scalar.activation(
                        sg_t[:, 0:width], ps_g[:, 0:width], AF.Silu)
                    nc.vector.tensor_tensor(
                        hs_sb[:, fc, 0:width], sg_t[:, 0:width],
                        ps_u[:, 0:width], op=AluOpType.mult)
                return hs_sb

            def down_sub(hs_sb, wd_sb, sub, w=P):
                yp = ypsum.tile([P, D], F32, tag="yp")
                dw = 512
                for half in range(D // dw):
                    for fc in range(FC):
                        nc.tensor.matmul(
                            yp[0:w, half * dw:(half + 1) * dw],
                            hs_sb[:, fc, sub * P:sub * P + w],
                            wd_sb[:, fc, half * dw:(half + 1) * dw],
                            start=(fc == 0), stop=(fc == FC - 1))
                return yp

            # ---- shared expert (unit E), streamed activations ----
            # first chunk is 256-wide so the PE starts ~3us earlier; the
            # first fc of the gate/up weights is interleaved between the
            # first two activation tiles on the SP ring for the same reason
            SH_CHUNKS = [(512 * i, 512) for i in range(NLOC // 512)]
            nsh = len(SH_CHUNKS)
            with ExitStack() as sctx:
                stpool = sctx.enter_context(
                    tc.tile_pool(name="stpool", bufs=2))

                def issue_xt(i):
                    s0, w = SH_CHUNKS[i]
                    t = stpool.tile([P, DC, 512], BF, tag="xt",
                                    name=f"xt_{_rep}_{i}")
                    nc.sync.dma_start(t[:, :, 0:w], xtb[:, :, s0:s0 + w])
                    return t

                # startup only: wu rides the idle Act ring so the fc
                # supply rate is 2x the PE's consumption rate (no
                # boundary-congestion hazard exists at t=0)
                xt_tiles = [issue_xt(0)]
                wgE = wpool.tile([P, FC, DC, P], BF, tag="wg")
                wuE = wpool.tile([P, FC, DC, P], BF, tag="wu")
                nc.sync.dma_start(wgE[:, 0], wg[E, 0])
                nc.sync.dma_start(wuE[:, 0], wu[E, 0])
                xt_tiles.append(issue_xt(1))
                for fc in range(1, FC):
                    nc.sync.dma_start(wgE[:, fc], wg[E, fc])
                    nc.sync.dma_start(wuE[:, fc], wu[E, fc])
                wdE = load_unit_d(E)
                w_next = None
                for i, (s0, w) in enumerate(SH_CHUNKS):
                    xcur = xt_tiles[i % 2]
                    hs_sb = gu_sweep(
                        wgE, wuE, lambda c, x=xcur, ww=w: x[:, c, 0:ww], w)
                    if i == 1:
                        # second chunk, not first: the startup DMA server
                        # must feed xtb + shared weights before anything else
                        w_next = load_unit_gu(EORDER[0])
                        xg_cur, cg_cur = issue_gathers(EORDER[0])
                    for sub in range(w // P):
                        yp = down_sub(hs_sb, wdE, sub)
                        ysh = spool.tile([P, D], F32, tag="ysh")
                        nc.scalar.copy(ysh[:], yp[:])
                        r0 = s0 + sub * P
                        nc.sync.dma_start(outp[r0:r0 + P, :], ysh[:])
                    if i + 2 < nsh:
                        xt_tiles[i % 2] = issue_xt(i + 2)
            wd_next = load_unit_d(EORDER[0])  # end-of-body: see load_unit_d

            # ---- routed experts, largest capacity first so the final
            # expert has the smallest tail scatter ----
            for ei in range(E):
                e = EORDER[ei]
                cap = CAPS[e]
                last = ei + 1 >= E
                wg_sb, wu_sb = w_next
                wd_sb = wd_next
                xg_sb, cg_sb = xg_cur, cg_cur
                ysc = ypool.tile([P, NSTMAX, D], F32, tag="ysc")
                first = True
                for (s0, sw) in _st_chunks(cap):
                    hs_sb = gu_sweep(
                        wg_sb, wu_sb,
                        lambda c, x=xg_sb, a=s0, b=sw: x[:, c, a:a + b], sw)
                    if first and not last:
                        # mid-body prefetch: issue after the first chunk so
                        # the slot-wait can't block queue heads at e's start
                        w_next = load_unit_gu(EORDER[ei + 1])
                        xg_cur, cg_cur = issue_gathers(EORDER[ei + 1])
                    nsub = -(-sw // P)
                    for sub in range(nsub):
                        w = min(P, sw - sub * P)
                        gsub = s0 // P + sub
                        yp = down_sub(hs_sb, wd_sb, sub, w=w)
                        nc.vector.tensor_scalar(
                            ysc[0:w, gsub, :], yp[0:w, :],
                            cg_sb[0:w, gsub, 0:1], None, op0=AluOpType.mult)
                    if first:
                        # scatter the first 256 slots early: spreads the
                        # RMW DMA off the expert boundary and shrinks the
                        # final drain tail
                        scA = nc.gpsimd.dma_scatter_add(
                            outp[:, :], ysc[:, 0:2, :], idx_sb[:, e, 0:16],
                            num_idxs=256, num_idxs_reg=cap_regs[256],
                            elem_size=D)
                        tile.add_dep_helper(scA.ins, lib_mlp.ins,
                                            reason="mlp lib")
                        first = False
                scB = nc.gpsimd.dma_scatter_add(
                    outp[:, :], ysc[:, 2:NSTS[e], :],
                    idx_sb[:, e, 16:CWS[e]],
                    num_idxs=cap - 256, num_idxs_reg=cap_regs[cap - 256],
                    elem_size=D)
                tile.add_dep_helper(scB.ins, lib_mlp.ins, reason="mlp lib")
                if not last:
                    wd_next = load_unit_d(EORDER[ei + 1])

    from concourse.library_overlay import lower_extended_insts
    lower_extended_insts(nc)
    if split_waits:
        _split_multi_waits(nc)
    return nc


# ---------------------------------------------------------------------------
# Host side


def _prep_weight_gu(w, DC, FC):
    # w [HALF, D] -> [FC, 128, DC, 128]: out[fc, p, c, f] = w[fc*128+f, c*128+p]
    wt = w.T.reshape(DC, P, FC, P).transpose(2, 1, 0, 3)
    return np.ascontiguousarray(wt.astype(bf16))


def _prep_weight_d(w, DC, FC):
    # w [D, HALF] -> [FC, 128, D]: out[fc, p, d] = w[d, fc*128+p]
    wt = w.T.reshape(FC, P, DC * P)
    return np.ascontiguousarray(wt.astype(bf16))


_BUILT = {}
_LAST_CAPS = None


def _get_built(key, **kw):
    if key not in _BUILT:
        _BUILT[key] = build_moe_hostroute(**kw)
    return _BUILT[key]


def _host_route(xf, gate_w, NCORES, NLOC, E):
    """fp64 routing + balanced token->core assignment.

    Routing (gate logits, top-2, softmax) runs in fp64 numpy. Tokens are
    then assigned to cores greedily to balance per-(core, expert) counts
    (penalizing any count crossing the 512 boundary, which would cost an
    extra 128-slot down-projection sub-tile), so per-expert capacities are
    minimal. Returns (perm, CAPS, idx_maps, combR_maps): core ci owns
    tokens perm[ci*NLOC:(ci+1)*NLOC].
    """
    N = xf.shape[0]
    logits = xf.astype(np.float64) @ gate_w.astype(np.float64).T   # [N, E]
    top2 = np.argsort(-logits, axis=1, kind='stable')[:, :2]       # [N, 2]
    tv = np.take_along_axis(logits, top2, axis=1)
    ex = np.exp(tv - tv[:, 0:1])
    w12 = ex / ex.sum(axis=1, keepdims=True)                       # [N, 2]

    # --- greedy balanced assignment ---
    glob = np.bincount(top2.ravel(), minlength=E)
    prio = np.maximum(glob[top2[:, 0]], glob[top2[:, 1]])
    order = np.argsort(-prio, kind='stable')
    counts = [[0] * E for _ in range(NCORES)]
    loads = [0] * NCORES
    assign = np.empty(N, dtype=np.int64)
    t2l = top2.tolist()
    for t in order.tolist():
        e1, e2 = t2l[t]
        best, bestscore = -1, None
        for c in range(NCORES):
            if loads[c] >= NLOC:
                continue
            cc = counts[c]
            n1, n2 = cc[e1] + 1, cc[e2] + 1
            score = ((n1 > 512) + (n2 > 512),
                     n1 if n1 > n2 else n2, n1 + n2, loads[c])
            if bestscore is None or score < bestscore:
                bestscore, best = score, c
        assign[t] = best
        counts[best][e1] += 1
        counts[best][e2] += 1
        loads[best] += 1
    perm = np.argsort(assign, kind='stable')

    counts = np.asarray(counts)
    CAPS = tuple(int(max(64, -(-counts[:, e].max() // 16) * 16))
                 for e in range(E))
    CWS = [c // 16 for c in CAPS]
    NSTMAX = max(-(-c // P) for c in CAPS)
    CWMAX = NSTMAX * 8          # gathers always fetch NSTMAX*128 entries

    idx_maps, combR_maps = [], []
    for ci in range(NCORES):
        toks = perm[ci * NLOC:(ci + 1) * NLOC]
        t2 = top2[toks]
        wl = w12[toks]
        idxa = np.full((P, E, CWMAX), NLOC, dtype=np.int16)
        cR = np.zeros((E, NLOC + 16, 64), dtype=np.float32)
        for e in range(E):
            rows, cols = np.nonzero(t2 == e)
            assert len(rows) <= CAPS[e], (e, len(rows), CAPS[e])
            cR[e, rows, :] = wl[rows, cols].astype(np.float32)[:, None]
            arr = np.full(CAPS[e], NLOC, dtype=np.int16)
            arr[:len(rows)] = rows.astype(np.int16)
            idci = arr.reshape(CWS[e], 16).T                        # [16, CW]
            idxa[:, e, :CWS[e]] = np.tile(idci, (8, 1))
        idx_maps.append(idxa)
        combR_maps.append(cR)
    return perm, CAPS, idx_maps, combR_maps


def prepare(x, gate_w, w_up, w_down, sg_gate, sg_up, sg_down):
    """Build (nc, in_maps, meta) for the 8-core SPMD launch."""
    global _LAST_CAPS
    B, T, D = x.shape
    E = gate_w.shape[0]
    FFN = w_up.shape[1]
    HALF = FFN // 2
    DC = D // P
    FC = HALF // P
    N = B * T
    NCORES = 8
    NLOC = N // NCORES

    xf = np.ascontiguousarray(x.reshape(N, D))
    perm, CAPS, idx_maps, combR_maps = _host_route(
        xf, gate_w, NCORES, NLOC, E)
    _LAST_CAPS = CAPS
    xf = xf[perm]

    nc = _get_built((DC, FC, E, NLOC, CAPS),
                    DC=DC, FC=FC, E=E, NLOC=NLOC, CAPS=CAPS)

    UNITS = E + 1
    wg_all = np.empty((UNITS, FC, P, DC, P), dtype=bf16)
    wu_all = np.empty((UNITS, FC, P, DC, P), dtype=bf16)
    wd_all = np.empty((UNITS, FC, P, D), dtype=bf16)
    for u in range(E):
        wg_all[u] = _prep_weight_gu(w_up[u, :HALF], DC, FC)
        wu_all[u] = _prep_weight_gu(w_up[u, HALF:], DC, FC)
        wd_all[u] = _prep_weight_d(w_down[u], DC, FC)
    wg_all[E] = _prep_weight_gu(sg_gate, DC, FC)
    wu_all[E] = _prep_weight_gu(sg_up, DC, FC)
    wd_all[E] = _prep_weight_d(sg_down, DC, FC)

    in_maps = []
    for ci in range(NCORES):
        xc = xf[ci * NLOC:(ci + 1) * NLOC]
        xt = xc.T.reshape(DC, P, NLOC).transpose(1, 0, 2)
        xtb = np.ascontiguousarray(xt.astype(bf16))
        xbp = np.zeros((NLOC + 16, D), dtype=bf16)
        xbp[:NLOC] = xc.astype(bf16)
        in_maps.append({
            "xtb": xtb, "xb": xbp,
            "wg": wg_all, "wu": wu_all, "wd": wd_all,
            "idx": idx_maps[ci], "combR": combR_maps[ci],
        })

    return nc, in_maps, (B, T, D, NLOC, NCORES, perm)


def postprocess(results, meta):
    B, T, D, NLOC, NCORES, perm = meta
    cat = np.concatenate(
        [results[ci]["out"][0:NLOC] for ci in range(NCORES)], axis=0)
    out = np.empty_like(cat)
    out[perm] = cat
    return out.reshape(B, T, D).astype(np.float32)


def kernel(x, gate_w, w_up, w_down, sg_gate, sg_up, sg_down):
    from concourse.bass_utils import run_bass_kernel_spmd

    nc, in_maps, meta = prepare(
        x, gate_w, w_up, w_down, sg_gate, sg_up, sg_down)
    r = run_bass_kernel_spmd(nc, in_maps, core_ids=list(range(meta[4])))
    return postprocess(r.results, meta)
